# revision 5
# baseline (speedup 1.0000x reference)
"""GroupWiseLinearProjector Trainium2 kernel.

Reference computation: x [B=16, C=768, H=64, W=64]; 16 spatial groups
(g = i*4+j owns pixels x[:, :, i::4, j::4]); per-group Linear
y_g = W_g @ x_pix + b_g with W_g [768, 768].

Strategy (8 NeuronCores, no cross-device comm):
  - Shard by GROUP: core c owns groups {2c, 2c+1}. Each core reads only
    its 2 weight matrices (resident in SBUF for the whole kernel) and
    exactly 1/8 of x / writes 1/8 of y -> minimal HBM traffic per core
    (~55 MB: 25.2 in + 25.2 out + 4.7 weights).
  - Host pre-gathers each group's strided pixels into dense feature-major
    slabs [C, 4096] so every device DMA moves >=2KB contiguous runs.
  - Device: per group a dense GEMM [768x768] @ [768x4096], tiled
    128x128x512, fp32 data fed to the PE as float32r (FP22 multiply,
    FP32 accumulate) -> full PE rate at N=512.
"""

import sys

for _p in ("/opt/trn_rl_repo", "/root/.axon_site/_ro/trn_rl_repo"):
    if _p not in sys.path:
        sys.path.append(_p)

import numpy as np

B, C, H, W, G = 16, 768, 64, 64, 16
N_CORES = 8
GPC = G // N_CORES  # groups per core = 2
P = 128
KT = C // P   # 6 contraction tiles
MT = C // P   # 6 output-channel tiles
NPIX = B * (H // 4) * (W // 4)  # 4096 pixels per group
NTILE = 512
NT = NPIX // NTILE  # 8 pixel tiles

# float32r: PE reads the fp32 bits, truncates to FP22 for the multiply,
# accumulates in FP32. 1 cycle/row at N>=256 (vs 4 for true fp32).
MM_DTYPE_NAME = "float32r"

_CACHE = {}


def _build_nc():
    import concourse.mybir as mybir
    import concourse.tile as tile
    from concourse import bacc

    f32 = mybir.dt.float32
    mm_dt = getattr(mybir.dt, MM_DTYPE_NAME)

    nc = bacc.Bacc(None, target_bir_lowering=False)
    # x and W carry fp32 bits but are declared with the matmul dtype so the
    # BIR verifier sees a consistent FP32r producer/consumer chain (numpy
    # side both map to np.float32).
    xg = nc.dram_tensor("xg", [GPC, C, NPIX], mm_dt, kind="ExternalInput")
    wt = nc.dram_tensor("wt", [GPC, C, C], mm_dt, kind="ExternalInput")  # wt[g] = W_g.T  [I, O]
    bias = nc.dram_tensor("bias", [GPC, C], f32, kind="ExternalInput")
    y = nc.dram_tensor("y", [GPC, C, NPIX], f32, kind="ExternalOutput")

    with tile.TileContext(nc) as tc:
        with (
            tc.tile_pool(name="wpool", bufs=1) as wpool,
            tc.tile_pool(name="bpool", bufs=1) as bpool,
            tc.tile_pool(name="xpool", bufs=3) as xpool,
            tc.tile_pool(name="opool", bufs=3) as opool,
            tc.tile_pool(name="pspool", bufs=8, space="PSUM") as pspool,
        ):
            # Both groups' transposed weights stay resident: 128 x (2*6*768) f32
            # = 36 KB/partition.
            w_tile = wpool.tile([P, GPC, KT, C], mm_dt)
            for g in range(GPC):
                nc.sync.dma_start(
                    w_tile[:, g], wt[g].rearrange("(ko p) m -> p ko m", p=P)
                )
            b_tile = bpool.tile([P, GPC, MT], f32)
            nc.sync.dma_start(b_tile[:], bias.rearrange("g (mo p) -> p g mo", p=P))

            for g in range(GPC):
                xv = xg[g].rearrange("(ko p) n -> p ko n", p=P)
                yv = y[g].rearrange("(mo p) n -> p mo n", p=P)
                for n in range(NT):
                    x_slab = xpool.tile([P, KT, NTILE], mm_dt, tag="x")
                    nc.sync.dma_start(
                        x_slab[:], xv[:, :, n * NTILE : (n + 1) * NTILE]
                    )
                    o_slab = opool.tile([P, MT, NTILE], f32, tag="o")
                    for m in range(MT):
                        ps = pspool.tile([P, NTILE], f32, tag="ps")
                        for k in range(KT):
                            nc.tensor.matmul(
                                ps[:],
                                w_tile[:, g, k, m * P : (m + 1) * P],
                                x_slab[:, k, :],
                                start=(k == 0),
                                stop=(k == KT - 1),
                            )
                        # PSUM -> SBUF eviction fused with the bias add.
                        nc.vector.tensor_scalar_add(
                            o_slab[:, m, :], ps[:], b_tile[:, g, m : m + 1]
                        )
                    nc.sync.dma_start(
                        yv[:, :, n * NTILE : (n + 1) * NTILE], o_slab[:]
                    )

    nc.compile()
    return nc


def _get_nc():
    if "nc" not in _CACHE:
        _CACHE["nc"] = _build_nc()
    return _CACHE["nc"]


def _shard_inputs(x, Wg, bg):
    """Host-side gather: group-major dense slabs, one in_map per core."""
    x = np.ascontiguousarray(np.asarray(x, dtype=np.float32))
    Wg = np.asarray(Wg, dtype=np.float32)
    bg = np.asarray(bg, dtype=np.float32)

    # xt[i, j, c, b, hh, ww] = x[b, c, 4*hh+i, 4*ww+j]; group g = i*4+j.
    xt = np.ascontiguousarray(
        x.reshape(B, C, H // 4, 4, W // 4, 4).transpose(3, 5, 1, 0, 2, 4)
    ).reshape(G, C, NPIX)
    wtT = np.ascontiguousarray(Wg.transpose(0, 2, 1))  # [G, I, O]

    in_maps = []
    for c in range(N_CORES):
        gs = slice(GPC * c, GPC * (c + 1))
        in_maps.append(
            {
                "xg": np.ascontiguousarray(xt[gs]),
                "wt": np.ascontiguousarray(wtT[gs]),
                "bias": np.ascontiguousarray(bg[gs]),
            }
        )
    return in_maps


def _unshard_output(results):
    """Host-side scatter of per-core [GPC, C, NPIX] back to [B, C, H, W]."""
    yt = np.empty((4, 4, C, B, H // 4, W // 4), np.float32)
    for c in range(N_CORES):
        yc = np.asarray(results[c]["y"]).reshape(GPC, C, B, H // 4, W // 4)
        for gl in range(GPC):
            g = GPC * c + gl
            yt[g // 4, g % 4] = yc[gl]
    # yt[i, j, c, b, hh, ww] -> y[b, c, 4*hh+i, 4*ww+j]
    return np.ascontiguousarray(yt.transpose(3, 2, 4, 0, 5, 1)).reshape(B, C, H, W)


def run(x, Wg, bg, trace=False):
    from concourse.bass_utils import run_bass_kernel_spmd

    nc = _get_nc()
    in_maps = _shard_inputs(x, Wg, bg)
    res = run_bass_kernel_spmd(
        nc, in_maps, core_ids=list(range(N_CORES)), trace=trace
    )
    return _unshard_output(res.results), res


def kernel(x, Wg, bg):
    out, _ = run(x, Wg, bg, trace=False)
    return out


# revision 8
# speedup vs baseline: 1.0385x; 1.0385x over previous
"""GroupWiseLinearProjector Trainium2 kernel.

Reference computation: x [B=16, C=768, H=64, W=64]; 16 spatial groups
(g = i*4+j owns pixels x[:, :, i::4, j::4]); per-group Linear
y_g = W_g @ x_pix + b_g with W_g [768, 768].

Strategy (8 NeuronCores, no cross-device comm):
  - Shard by GROUP: core c owns groups {2c, 2c+1}. Each core reads only
    its 2 weight matrices (resident in SBUF for the whole kernel) and
    exactly 1/8 of x / writes 1/8 of y -> minimal HBM traffic per core
    (~55 MB: 25.2 in + 25.2 out + 4.7 weights).
  - Host pre-gathers each group's strided pixels into dense feature-major
    slabs [C, 4096] so every device DMA moves >=2KB contiguous runs.
  - Device: per group a dense GEMM [768x768] @ [768x4096], tiled
    128x128x512, fp32 data fed to the PE as float32r (FP22 multiply,
    FP32 accumulate) -> full PE rate at N=512.
  - DMA granularity tuned for overlap: weights load in per-(g,k) chunks,
    x in per-k chunks of n-slabs (first slabs narrow to fill the pipe
    fast, then 1024-wide for 4KB DMA runs), outputs stored per-m so
    stores trail each eviction instead of each slab.
  - PSUM->SBUF eviction (fused bias add) alternates Vector/Scalar
    engines so neither becomes the tail.
"""

import sys

for _p in ("/opt/trn_rl_repo", "/root/.axon_site/_ro/trn_rl_repo"):
    if _p not in sys.path:
        sys.path.append(_p)

import numpy as np

B, C, H, W, G = 16, 768, 64, 64, 16
N_CORES = 8
GPC = G // N_CORES  # groups per core = 2
P = 128
KT = C // P   # 6 contraction tiles
MT = C // P   # 6 output-channel tiles
NPIX = B * (H // 4) * (W // 4)  # 4096 pixels per group
MMN = 512  # matmul moving free dim (one PSUM bank of fp32)

# n-slab widths per group: narrow at the very start (fast pipeline fill)
# and at the very end (short tail), wide in steady state (4KB DMA runs).
SLABS = {
    0: [512, 512, 1024, 1024, 1024],
    1: [1024, 1024, 1024, 512, 512],
}

MM_DTYPE_NAME = "float32r"
USE_ACT_EVICT = False

_CACHE = {}


def _build_nc():
    import concourse.mybir as mybir
    import concourse.tile as tile
    from concourse import bacc

    f32 = mybir.dt.float32
    mm_dt = getattr(mybir.dt, MM_DTYPE_NAME)
    act = mybir.ActivationFunctionType

    nc = bacc.Bacc(None, target_bir_lowering=False)
    # x and W carry fp32 bits but are declared with the matmul dtype so the
    # BIR verifier sees a consistent FP32r producer/consumer chain (numpy
    # side both map to np.float32).
    xg = nc.dram_tensor("xg", [GPC, C, NPIX], mm_dt, kind="ExternalInput")
    wt = nc.dram_tensor("wt", [GPC, C, C], mm_dt, kind="ExternalInput")  # wt[g] = W_g.T  [I, O]
    bias = nc.dram_tensor("bias", [GPC, C], f32, kind="ExternalInput")
    y = nc.dram_tensor("y", [GPC, C, NPIX], f32, kind="ExternalOutput")

    with tile.TileContext(nc) as tc:
        with (
            tc.tile_pool(name="wpool", bufs=1) as wpool,
            tc.tile_pool(name="bpool", bufs=1) as bpool,
            tc.tile_pool(name="xpool", bufs=3) as xpool,
            tc.tile_pool(name="opool", bufs=2) as opool,
            tc.tile_pool(name="pspool", bufs=8, space="PSUM") as pspool,
        ):
            # Both groups' transposed weights stay resident: 128 x (2*6*768)
            # f32 = 36 KB/partition. Loaded in per-(g,k) 384KB chunks so the
            # first matmuls aren't gated on the full 4.7MB.
            w_tile = wpool.tile([P, GPC, KT, C], mm_dt)
            b_tile = bpool.tile([P, GPC, MT], f32)
            nc.sync.dma_start(b_tile[:], bias.rearrange("g (mo p) -> p g mo", p=P))
            for g in range(GPC):
                for k in range(KT):
                    nc.sync.dma_start(
                        w_tile[:, g, k], wt[g, k * P : (k + 1) * P, :]
                    )

            for g in range(GPC):
                xv = xg[g].rearrange("(ko p) n -> p ko n", p=P)
                yv = y[g].rearrange("(mo p) n -> p mo n", p=P)
                n0 = 0
                for width in SLABS[g]:
                    x_slab = xpool.tile([P, KT, width], mm_dt, tag="x")
                    # Per-k chunks: matmuls on chunk k start as soon as it
                    # lands instead of waiting for the whole slab.
                    for k in range(KT):
                        nc.sync.dma_start(
                            x_slab[:, k, :], xv[:, k, n0 : n0 + width]
                        )
                    o_slab = opool.tile([P, MT, width], f32, tag="o")
                    for h in range(width // MMN):
                        hs = slice(h * MMN, (h + 1) * MMN)
                        for m in range(MT):
                            ps = pspool.tile([P, MMN], f32, tag="ps")
                            for k in range(KT):
                                nc.tensor.matmul(
                                    ps[:],
                                    w_tile[:, g, k, m * P : (m + 1) * P],
                                    x_slab[:, k, hs],
                                    start=(k == 0),
                                    stop=(k == KT - 1),
                                )
                            # PSUM -> SBUF eviction fused with the bias add,
                            # alternating engines.
                            if USE_ACT_EVICT and (m + h) % 2 == 1:
                                nc.scalar.activation(
                                    o_slab[:, m, hs],
                                    ps[:],
                                    act.Identity,
                                    bias=b_tile[:, g, m : m + 1],
                                )
                            else:
                                nc.vector.tensor_scalar_add(
                                    o_slab[:, m, hs], ps[:], b_tile[:, g, m : m + 1]
                                )
                    # Per-m stores trail the evictions.
                    for m in range(MT):
                        nc.sync.dma_start(
                            yv[:, m, n0 : n0 + width], o_slab[:, m, :]
                        )
                    n0 += width

    nc.compile()
    return nc


def _get_nc():
    if "nc" not in _CACHE:
        _CACHE["nc"] = _build_nc()
    return _CACHE["nc"]


def _shard_inputs(x, Wg, bg):
    """Host-side gather: group-major dense slabs, one in_map per core."""
    x = np.ascontiguousarray(np.asarray(x, dtype=np.float32))
    Wg = np.asarray(Wg, dtype=np.float32)
    bg = np.asarray(bg, dtype=np.float32)

    # xt[i, j, c, b, hh, ww] = x[b, c, 4*hh+i, 4*ww+j]; group g = i*4+j.
    xt = np.ascontiguousarray(
        x.reshape(B, C, H // 4, 4, W // 4, 4).transpose(3, 5, 1, 0, 2, 4)
    ).reshape(G, C, NPIX)
    wtT = np.ascontiguousarray(Wg.transpose(0, 2, 1))  # [G, I, O]

    in_maps = []
    for c in range(N_CORES):
        gs = slice(GPC * c, GPC * (c + 1))
        in_maps.append(
            {
                "xg": np.ascontiguousarray(xt[gs]),
                "wt": np.ascontiguousarray(wtT[gs]),
                "bias": np.ascontiguousarray(bg[gs]),
            }
        )
    return in_maps


def _unshard_output(results):
    """Host-side scatter of per-core [GPC, C, NPIX] back to [B, C, H, W]."""
    yt = np.empty((4, 4, C, B, H // 4, W // 4), np.float32)
    for c in range(N_CORES):
        yc = np.asarray(results[c]["y"]).reshape(GPC, C, B, H // 4, W // 4)
        for gl in range(GPC):
            g = GPC * c + gl
            yt[g // 4, g % 4] = yc[gl]
    # yt[i, j, c, b, hh, ww] -> y[b, c, 4*hh+i, 4*ww+j]
    return np.ascontiguousarray(yt.transpose(3, 2, 4, 0, 5, 1)).reshape(B, C, H, W)


def run(x, Wg, bg, trace=False):
    from concourse.bass_utils import run_bass_kernel_spmd

    nc = _get_nc()
    in_maps = _shard_inputs(x, Wg, bg)
    res = run_bass_kernel_spmd(
        nc, in_maps, core_ids=list(range(N_CORES)), trace=trace
    )
    return _unshard_output(res.results), res


def kernel(x, Wg, bg):
    out, _ = run(x, Wg, bg, trace=False)
    return out


# revision 9
# speedup vs baseline: 1.1020x; 1.0611x over previous
"""GroupWiseLinearProjector Trainium2 kernel.

Reference computation: x [B=16, C=768, H=64, W=64]; 16 spatial groups
(g = i*4+j owns pixels x[:, :, i::4, j::4]); per-group Linear
y_g = W_g @ x_pix + b_g with W_g [768, 768].

Strategy (8 NeuronCores, no cross-device comm):
  - Shard by GROUP: core c owns groups {2c, 2c+1}. Each core reads only
    its 2 weight matrices (resident in SBUF for the whole kernel) and
    exactly 1/8 of x / writes 1/8 of y -> minimal HBM traffic per core
    (~55 MB: 25.2 in + 25.2 out + 4.7 weights).
  - Host pre-gathers each group's strided pixels into dense feature-major
    slabs [C, 4096] so every device DMA moves >=2KB contiguous runs.
  - Device: per group a dense GEMM [768x768] @ [768x4096], tiled
    128x128x512, fp32 data fed to the PE as float32r (FP22 multiply,
    FP32 accumulate) -> full PE rate at N=512.
  - Overlap structure: weights stream in per-(g,k) chunks interleaved
    with the first x slab so the PE starts within ~3us; x slabs are
    software-prefetched one ahead in per-k chunks; m-outer compute lets
    each output row-block store while later blocks still compute; loads
    issue from the Sync HWDGE queue, stores from the Scalar HWDGE queue
    so neither FIFO blocks the other.
"""

import sys

for _p in ("/opt/trn_rl_repo", "/root/.axon_site/_ro/trn_rl_repo"):
    if _p not in sys.path:
        sys.path.append(_p)

import numpy as np

B, C, H, W, G = 16, 768, 64, 64, 16
N_CORES = 8
GPC = G // N_CORES  # groups per core = 2
P = 128
KT = C // P   # 6 contraction tiles
MT = C // P   # 6 output-channel tiles
NPIX = B * (H // 4) * (W // 4)  # 4096 pixels per group
MMN = 512  # matmul moving free dim (one PSUM bank of fp32)

# n-slab widths: narrow at the start (fast pipeline fill while weights
# stream) and at the end (short drain), 1024-wide in steady state (4KB
# DMA runs; the x-prefetch deficit stays small enough that PE stalls are
# short and HAM stays warm).
SLAB_WIDTHS = (
    [(0, 512), (0, 512), (0, 1024), (0, 1024), (0, 1024)]
    + [(1, 1024), (1, 1024), (1, 1024), (1, 512), (1, 512)]
)

MM_DTYPE_NAME = "float32r"

_CACHE = {}


def _build_nc():
    import concourse.mybir as mybir
    import concourse.tile as tile
    from concourse import bacc

    f32 = mybir.dt.float32
    mm_dt = getattr(mybir.dt, MM_DTYPE_NAME)

    nc = bacc.Bacc(None, target_bir_lowering=False)
    # x and W carry fp32 bits but are declared with the matmul dtype so the
    # BIR verifier sees a consistent FP32r producer/consumer chain (numpy
    # side both map to np.float32).
    xg = nc.dram_tensor("xg", [GPC, C, NPIX], mm_dt, kind="ExternalInput")
    wt = nc.dram_tensor("wt", [GPC, C, C], mm_dt, kind="ExternalInput")  # wt[g] = W_g.T  [I, O]
    # bias pre-arranged on host to [128, GPC*MT] so its DMA is dense rows,
    # not a 4-byte-per-element gather.
    bias = nc.dram_tensor("bias", [P, GPC * MT], f32, kind="ExternalInput")
    y = nc.dram_tensor("y", [GPC, C, NPIX], f32, kind="ExternalOutput")

    # slab start offsets
    slabs = []
    n0 = {0: 0, 1: 0}
    for g, wdt in SLAB_WIDTHS:
        slabs.append((g, n0[g], wdt))
        n0[g] += wdt
    assert n0 == {0: NPIX, 1: NPIX}

    xv = [xg[g].rearrange("(ko p) n -> p ko n", p=P) for g in range(GPC)]
    yv = [y[g].rearrange("(mo p) n -> p mo n", p=P) for g in range(GPC)]

    with tile.TileContext(nc) as tc:
        with (
            tc.tile_pool(name="wpool", bufs=1) as wpool,
            tc.tile_pool(name="bpool", bufs=1) as bpool,
            tc.tile_pool(name="xpool", bufs=3) as xpool,
            tc.tile_pool(name="opool", bufs=6) as opool,
            tc.tile_pool(name="pspool", bufs=8, space="PSUM") as pspool,
        ):
            w_tile = wpool.tile([P, GPC, KT, C], mm_dt)  # 36 KB/partition
            b_tile = bpool.tile([P, GPC, MT], f32)
            nc.sync.dma_start(b_tile[:], bias.rearrange("p (g mo) -> p g mo", g=GPC))

            def load_slab(i):
                g, s0, wdt = slabs[i]
                t = xpool.tile([P, KT, wdt], mm_dt, tag="x")
                for k in range(KT):
                    nc.sync.dma_start(t[:, k, :], xv[g][:, k, s0 : s0 + wdt])
                return t

            # Startup: group-0 weight chunks interleaved with slab 0's
            # chunks so the first matmul is gated on ~0.6MB, not 4.7MB.
            g0_slab = xpool.tile([P, KT, slabs[0][2]], mm_dt, tag="x")
            for k in range(KT):
                nc.sync.dma_start(w_tile[:, 0, k], wt[0, k * P : (k + 1) * P, :])
                nc.sync.dma_start(g0_slab[:, k, :], xv[0][:, k, : slabs[0][2]])

            x_tiles = {0: g0_slab}
            for i, (g, s0, wdt) in enumerate(slabs):
                # Prefetch the next slab ahead of this slab's compute; its
                # chunks sit ahead of everything later on the load queue.
                if i + 1 < len(slabs):
                    x_tiles[i + 1] = load_slab(i + 1)
                # Stream group-1 weight chunks one per early iteration
                # (needed from slab 5 on; chunk 0 arrives by slab 1).
                if 0 <= i < KT:
                    nc.sync.dma_start(
                        w_tile[:, 1, i], wt[1, i * P : (i + 1) * P, :]
                    )
                x_slab = x_tiles.pop(i)
                for m in range(MT):
                    o_m = opool.tile([P, wdt], f32, tag="o")
                    for h in range(wdt // MMN):
                        hs = slice(h * MMN, (h + 1) * MMN)
                        ps = pspool.tile([P, MMN], f32, tag="ps")
                        for k in range(KT):
                            nc.tensor.matmul(
                                ps[:],
                                w_tile[:, g, k, m * P : (m + 1) * P],
                                x_slab[:, k, hs],
                                start=(k == 0),
                                stop=(k == KT - 1),
                            )
                        # PSUM -> SBUF eviction fused with the bias add.
                        nc.vector.tensor_scalar_add(
                            o_m[:, hs], ps[:], b_tile[:, g, m : m + 1]
                        )
                    # Store this row-block now (Scalar HWDGE queue, so
                    # stores never block later slab loads on Sync's queue).
                    nc.scalar.dma_start(yv[g][:, m, s0 : s0 + wdt], o_m[:])

    nc.compile()
    return nc


def _get_nc():
    if "nc" not in _CACHE:
        _CACHE["nc"] = _build_nc()
    return _CACHE["nc"]


def _shard_inputs(x, Wg, bg):
    """Host-side gather: group-major dense slabs, one in_map per core."""
    x = np.ascontiguousarray(np.asarray(x, dtype=np.float32))
    Wg = np.asarray(Wg, dtype=np.float32)
    bg = np.asarray(bg, dtype=np.float32)

    # xt[i, j, c, b, hh, ww] = x[b, c, 4*hh+i, 4*ww+j]; group g = i*4+j.
    xt = np.ascontiguousarray(
        x.reshape(B, C, H // 4, 4, W // 4, 4).transpose(3, 5, 1, 0, 2, 4)
    ).reshape(G, C, NPIX)
    wtT = np.ascontiguousarray(Wg.transpose(0, 2, 1))  # [G, I, O]
    # bias[g, mo*128+p] -> [p, g*MT+mo]
    bias_arr = bg.reshape(G, MT, P).transpose(2, 0, 1)  # [P, G, MT]

    in_maps = []
    for c in range(N_CORES):
        gs = slice(GPC * c, GPC * (c + 1))
        in_maps.append(
            {
                "xg": np.ascontiguousarray(xt[gs]),
                "wt": np.ascontiguousarray(wtT[gs]),
                "bias": np.ascontiguousarray(
                    bias_arr[:, gs, :].reshape(P, GPC * MT)
                ),
            }
        )
    return in_maps


def _unshard_output(results):
    """Host-side scatter of per-core [GPC, C, NPIX] back to [B, C, H, W]."""
    yt = np.empty((4, 4, C, B, H // 4, W // 4), np.float32)
    for c in range(N_CORES):
        yc = np.asarray(results[c]["y"]).reshape(GPC, C, B, H // 4, W // 4)
        for gl in range(GPC):
            g = GPC * c + gl
            yt[g // 4, g % 4] = yc[gl]
    # yt[i, j, c, b, hh, ww] -> y[b, c, 4*hh+i, 4*ww+j]
    return np.ascontiguousarray(yt.transpose(3, 2, 4, 0, 5, 1)).reshape(B, C, H, W)


def run(x, Wg, bg, trace=False):
    from concourse.bass_utils import run_bass_kernel_spmd

    nc = _get_nc()
    in_maps = _shard_inputs(x, Wg, bg)
    res = run_bass_kernel_spmd(
        nc, in_maps, core_ids=list(range(N_CORES)), trace=trace
    )
    return _unshard_output(res.results), res


def kernel(x, Wg, bg):
    out, _ = run(x, Wg, bg, trace=False)
    return out


# revision 11
# speedup vs baseline: 1.2572x; 1.1409x over previous
"""GroupWiseLinearProjector Trainium2 kernel.

Reference computation: x [B=16, C=768, H=64, W=64]; 16 spatial groups
(g = i*4+j owns pixels x[:, :, i::4, j::4]); per-group Linear
y_g = W_g @ x_pix + b_g with W_g [768, 768].

Strategy (8 NeuronCores, no cross-device comm):
  - Shard by GROUP: core c owns groups {2c, 2c+1}. Each core reads only
    its 2 weight matrices (resident in SBUF for the whole kernel) and
    exactly 1/8 of x / writes 1/8 of y -> minimal HBM traffic per core
    (~55 MB: 25.2 in + 25.2 out + 4.7 weights).
  - Host pre-gathers each group's strided pixels into SLAB-MAJOR layout
    [slab, partition, k, 1024] so a whole 3MB x-slab (and each y-slab
    store) moves with 24KB-per-partition contiguous DMA descriptors --
    the per-descriptor overhead is what caps effective HBM bandwidth.
  - Device: per group a dense GEMM [768x768] @ [768x4096], tiled
    128x128x512, fp32 data fed to the PE as float32r (FP22 multiply,
    FP32 accumulate) -> full PE rate at N=512. Loop order m->k->h so one
    stationary weight load feeds 2 matmuls.
  - Overlap: per-k chunk loads for the first two slabs (PE starts within
    ~3us and rides chunk arrivals), whole-slab single DMAs afterwards;
    x prefetched one slab ahead; loads on the Sync HWDGE queue, stores
    on the Scalar HWDGE queue so neither FIFO blocks the other.
"""

import sys

for _p in ("/opt/trn_rl_repo", "/root/.axon_site/_ro/trn_rl_repo"):
    if _p not in sys.path:
        sys.path.append(_p)

import numpy as np

B, C, H, W, G = 16, 768, 64, 64, 16
N_CORES = 8
GPC = G // N_CORES  # groups per core = 2
P = 128
KT = C // P   # 6 contraction tiles
MT = C // P   # 6 output-channel tiles
NPIX = B * (H // 4) * (W // 4)  # 4096 pixels per group
MMN = 512     # matmul moving free dim (one PSUM bank of fp32)
WIDTH = 1024  # n-slab width
SPG = NPIX // WIDTH  # 4 slabs per group
NSLAB = GPC * SPG    # 8 slabs per core
HB = WIDTH // MMN    # 2 psum banks per (m) block

MM_DTYPE_NAME = "float32r"

_CACHE = {}


def _build_nc():
    import concourse.mybir as mybir
    import concourse.tile as tile
    from concourse import bacc

    f32 = mybir.dt.float32
    mm_dt = getattr(mybir.dt, MM_DTYPE_NAME)

    nc = bacc.Bacc(None, target_bir_lowering=False)
    # x and W carry fp32 bits but are declared with the matmul dtype so the
    # BIR verifier sees a consistent FP32r producer/consumer chain (numpy
    # side both map to np.float32).
    xg = nc.dram_tensor("xg", [NSLAB, P, KT, WIDTH], mm_dt, kind="ExternalInput")
    wt = nc.dram_tensor("wt", [GPC, KT, P, C], mm_dt, kind="ExternalInput")
    bias = nc.dram_tensor("bias", [P, GPC * MT], f32, kind="ExternalInput")
    y = nc.dram_tensor("y", [NSLAB, P, MT, WIDTH], f32, kind="ExternalOutput")

    with tile.TileContext(nc) as tc:
        with (
            tc.tile_pool(name="wpool", bufs=1) as wpool,
            tc.tile_pool(name="bpool", bufs=1) as bpool,
            tc.tile_pool(name="xpool", bufs=3) as xpool,
            tc.tile_pool(name="opool", bufs=2) as opool,
            tc.tile_pool(name="pspool", bufs=8, space="PSUM") as pspool,
        ):
            w_tile = wpool.tile([P, GPC, KT, C], mm_dt)  # 36 KB/partition
            b_tile = bpool.tile([P, GPC, MT], f32)
            nc.sync.dma_start(b_tile[:], bias.rearrange("p (g mo) -> p g mo", g=GPC))

            def load_slab(i, chunked):
                t = xpool.tile([P, KT, WIDTH], mm_dt, tag="x")
                if chunked:
                    for k in range(KT):
                        nc.sync.dma_start(t[:, k, :], xg[i, :, k, :])
                else:
                    nc.sync.dma_start(t[:], xg[i])
                return t

            # Startup: group-0 weight chunks interleaved with slab 0's
            # chunks so the first matmul is gated on ~0.9MB, not 4.7MB.
            slab0 = xpool.tile([P, KT, WIDTH], mm_dt, tag="x")
            for k in range(KT):
                nc.sync.dma_start(w_tile[:, 0, k], wt[0, k])
                nc.sync.dma_start(slab0[:, k, :], xg[0, :, k, :])

            x_tiles = {0: slab0}
            for i in range(NSLAB):
                g = i // SPG
                if i + 1 < NSLAB:
                    x_tiles[i + 1] = load_slab(i + 1, chunked=(i + 1 < 2))
                # Stream group-1 weight chunks during the first iterations
                # (all landed long before slab 4 needs them).
                if GPC > 1 and i < 3:
                    nc.sync.dma_start(w_tile[:, 1, 2 * i], wt[1, 2 * i])
                    nc.sync.dma_start(w_tile[:, 1, 2 * i + 1], wt[1, 2 * i + 1])
                x_slab = x_tiles.pop(i)
                o_slab = opool.tile([P, MT, WIDTH], f32, tag="o")
                last = i == NSLAB - 1
                for m in range(MT):
                    pss = [
                        pspool.tile([P, MMN], f32, tag="ps", name=f"ps_{i}_{m}_{h}")
                        for h in range(HB)
                    ]
                    for k in range(KT):
                        for h in range(HB):
                            # One stationary weight load feeds HB matmuls.
                            nc.tensor.matmul(
                                pss[h][:],
                                w_tile[:, g, k, m * P : (m + 1) * P],
                                x_slab[:, k, h * MMN : (h + 1) * MMN],
                                start=(k == 0),
                                stop=(k == KT - 1),
                            )
                    for h in range(HB):
                        # PSUM -> SBUF eviction fused with the bias add.
                        nc.vector.tensor_scalar_add(
                            o_slab[:, m, h * MMN : (h + 1) * MMN],
                            pss[h][:],
                            b_tile[:, g, m : m + 1],
                        )
                    if last:
                        # Per-m stores on the final slab shorten the drain.
                        nc.scalar.dma_start(y[i, :, m, :], o_slab[:, m, :])
                if not last:
                    # Whole-slab store: 24KB contiguous per partition.
                    nc.scalar.dma_start(y[i], o_slab[:])

    nc.compile()
    return nc


def _get_nc():
    if "nc" not in _CACHE:
        _CACHE["nc"] = _build_nc()
    return _CACHE["nc"]


def _shard_inputs(x, Wg, bg):
    """Host-side gather: slab-major dense blocks, one in_map per core."""
    x = np.ascontiguousarray(np.asarray(x, dtype=np.float32))
    Wg = np.asarray(Wg, dtype=np.float32)
    bg = np.asarray(bg, dtype=np.float32)

    # xt[i, j, c, b, hh, ww] = x[b, c, 4*hh+i, 4*ww+j]; group g = i*4+j.
    xt = np.ascontiguousarray(
        x.reshape(B, C, H // 4, 4, W // 4, 4).transpose(3, 5, 1, 0, 2, 4)
    ).reshape(G, C, NPIX)
    # [G, KT, P, SPG, WIDTH] -> slab-major [G, SPG, P, KT, WIDTH]
    xs = np.ascontiguousarray(
        xt.reshape(G, KT, P, SPG, WIDTH).transpose(0, 3, 2, 1, 4)
    )
    # wt[g, k, p, m] = Wg[g, m, k*128+p]
    wtT = np.ascontiguousarray(
        Wg.transpose(0, 2, 1).reshape(G, KT, P, C)
    )
    bias_arr = bg.reshape(G, MT, P).transpose(2, 0, 1)  # [P, G, MT]

    in_maps = []
    for c in range(N_CORES):
        gs = slice(GPC * c, GPC * (c + 1))
        in_maps.append(
            {
                "xg": xs[gs].reshape(NSLAB, P, KT, WIDTH),
                "wt": np.ascontiguousarray(wtT[gs]),
                "bias": np.ascontiguousarray(
                    bias_arr[:, gs, :].reshape(P, GPC * MT)
                ),
            }
        )
    return in_maps


def _unshard_output(results):
    """Host-side scatter of per-core [NSLAB, P, MT, WIDTH] to [B, C, H, W]."""
    yt = np.empty((4, 4, C, B, H // 4, W // 4), np.float32)
    for c in range(N_CORES):
        # [GPC, SPG, P, MT, WIDTH] -> [GPC, MT, P, SPG*WIDTH]
        yc = (
            np.asarray(results[c]["y"])
            .reshape(GPC, SPG, P, MT, WIDTH)
            .transpose(0, 3, 2, 1, 4)
            .reshape(GPC, C, B, H // 4, W // 4)
        )
        for gl in range(GPC):
            g = GPC * c + gl
            yt[g // 4, g % 4] = yc[gl]
    # yt[i, j, c, b, hh, ww] -> y[b, c, 4*hh+i, 4*ww+j]
    return np.ascontiguousarray(yt.transpose(3, 2, 4, 0, 5, 1)).reshape(B, C, H, W)


def run(x, Wg, bg, trace=False):
    from concourse.bass_utils import run_bass_kernel_spmd

    nc = _get_nc()
    in_maps = _shard_inputs(x, Wg, bg)
    res = run_bass_kernel_spmd(
        nc, in_maps, core_ids=list(range(N_CORES)), trace=trace
    )
    return _unshard_output(res.results), res


def kernel(x, Wg, bg):
    out, _ = run(x, Wg, bg, trace=False)
    return out
